# revision 40
# baseline (speedup 1.0000x reference)
"""Trainium2 Bass kernel: weighted BCE + IoU loss (structure loss).

Full inputs: pred/mask [64, 1, 512, 512] fp32.  Data-parallel over 8
NeuronCores (8 images per core).  Per image the device computes
per-partition partial sums of
  a = |box31(M)/961 - M|,  sp = ln(1+E),  ratio = numB/den,
  a*t4,  P*M
with  E = e^P, numB = 1+E+E*M, den = 1+M+2E,
      t4 = (sp - P*M) - ratio  (= bce + iou - 1).
Host finishes:  sum(weight*t)/sum(weight) with weight = 1+5a via
  swt = S(a*t4) + 0.2*S(t4) + S(a) + 0.2*HW,  S(t4) = S(sp)-S(pm)-S(ratio)
  loss_img = 5*swt / (HW + 5*S(a)),  output = mean over 64 images.

Structure (V3):
- Box pass 1 (H direction) = banded {0,1}-matmul W1 = B @ M on PE,
  contracting the partition dim -> NO input transpose.
- Box pass 2 (W direction) = fp32 prefix-sum scan along the free dim
  (tensor_tensor_scan reads W1 straight from PSUM) + shifted-difference
  TTs: box[w] = S[w+15] - S[w-16]; segment edges via stride-0 broadcast
  reads of S[j,511].  No transposes, no PSUM evacuation copies.
- xm = box - 961*M (plain 2x TT); a = |xm|/961 via the Abs scale with a
  free per-partition accum (sum a).
- ratio = (hen+0.5)*rcp3 where rcp3 = exp(-ln(den3)): division as one
  Ln + one Exp, ratio via one stt (its accum gives sum ratio free).
- sum(a*t4)/sum(pm) via 16 accumulated ones-matmuls each on the idle PE
  (per-partition strided colsums; host adds the final partition sum).
- Plain TTs are split DVE/Pool(GpSimd) for balance (K_* env knobs);
  the B-phase of image i (xm/Abs/at4/colsums) is emitted after image
  i+1's scan so DVE never head-blocks on Pool's Sdiff.
"""

import os as _os
from contextlib import ExitStack

import numpy as np

_B = 64
_H = 512
_W = 512
_NC = 8
_BPC = _B // _NC  # images per core
_HW = float(_H * _W)
_KHALF = 15  # box filter half width (31 taps)

_CACHE = {}

# tuning knobs (resolved at build time; K_* env overrides).  K_SP_x = how
# many of the op's 4 j-chunks run on Pool/GpSimd (rest on DVE).
_SP_DEN3 = int(_os.environ.get("K_SP_DEN3", "0"))
_SP_HEN = int(_os.environ.get("K_SP_HEN", "1"))
_SP_PM = int(_os.environ.get("K_SP_PM", "4"))
_SP_T1 = int(_os.environ.get("K_SP_T1", "2"))
_SP_T4 = int(_os.environ.get("K_SP_T4", "0"))
_SDIFF_POOL = _os.environ.get("K_SDIFF_POOL", "1") == "1"
_MBUFS = int(_os.environ.get("K_MBUFS", "3"))
_PBUFS = int(_os.environ.get("K_PBUFS", "1"))
_TAIL_IMGS = int(_os.environ.get("K_TAIL_IMGS", "0"))
_SDIFF_TAIL_DVE = _os.environ.get("K_SDIFF_TAIL_DVE", "1") == "1"
_TT_TAIL = _os.environ.get("K_TT_TAIL", "0") == "1"
_SD_COLS = int(_os.environ.get("K_SD_COLS", "481"))


def _band_np():
    import ml_dtypes

    idx = np.arange(_H)
    b = (np.abs(idx[:, None] - idx[None, :]) <= _KHALF).astype(np.float32)
    return b.astype(ml_dtypes.bfloat16)


def _pin_act_table_set():
    """Keep every activation in natural_log_exp_and_others (has Exp, Ln,
    Abs, Copy, Identity) so the kernel needs exactly one ACT table load."""
    import concourse.bacc as bacc_mod
    import concourse.bass_interp as interp_mod
    from concourse.hw_specs import get_activation_tables as real_gat

    keep = "natural_log_exp_and_others"

    def patched(arch):
        t = real_gat(arch)
        return {k: (v if k == keep else set()) for k, v in t.items()}

    bacc_mod.get_activation_tables = patched
    interp_mod.get_activation_tables = patched


def _build():
    if "nc" in _CACHE:
        return _CACHE["nc"]

    import concourse.bass as bass
    import concourse.tile as tile
    from concourse import bacc, mybir

    _pin_act_table_set()

    AF = mybir.ActivationFunctionType
    ALU = mybir.AluOpType
    F32 = mybir.dt.float32
    BF16 = mybir.dt.bfloat16
    ts = bass.ts

    nc = bacc.Bacc(
        "TRN2", target_bir_lowering=False, debug=False, num_devices=_NC
    )

    pred_d = nc.dram_tensor("pred", [_BPC, _H, _W], BF16, kind="ExternalInput").ap()
    mask_d = nc.dram_tensor("mask", [_BPC, _H, _W], BF16, kind="ExternalInput").ap()
    mh_d = nc.dram_tensor("mh", [_BPC, _H, _W], BF16, kind="ExternalInput").ap()
    m961_d = nc.dram_tensor("m961", [_BPC, _H, _W], BF16, kind="ExternalInput").ap()
    band_d = nc.dram_tensor("band", [_H, _W], BF16, kind="ExternalInput").ap()
    ones_d = nc.dram_tensor("ones", [128, 1], BF16, kind="ExternalInput").ap()
    out_d = nc.dram_tensor("out", [128, 5 * _BPC], F32, kind="ExternalOutput").ap()

    KH = _KHALF  # 15
    KW = 2 * _KHALF + 1  # 31

    def tt_split(pool_n, out_t, a_t, b_t, alu, img=0):
        """Run a [128,4,512] TT op: first pool_n j-chunks on gpsimd, rest DVE."""
        if _TT_TAIL and img >= _BPC - _TAIL_IMGS:
            pool_n = 0
        if pool_n > 0:
            nc.gpsimd.tensor_tensor(
                out_t[:, 0:pool_n, :], a_t[:, 0:pool_n, :], b_t[:, 0:pool_n, :], alu
            )
        if pool_n < 4:
            nc.vector.tensor_tensor(
                out_t[:, pool_n:4, :], a_t[:, pool_n:4, :], b_t[:, pool_n:4, :], alu
            )

    with tile.TileContext(nc) as tc, ExitStack() as ctx:
        cpool = ctx.enter_context(tc.tile_pool(name="cpool", bufs=1))
        ipool = ctx.enter_context(tc.tile_pool(name="ipool", bufs=2))
        mpool = ctx.enter_context(tc.tile_pool(name="mpool", bufs=_MBUFS))
        spool = ctx.enter_context(tc.tile_pool(name="spool", bufs=2))
        pw = ctx.enter_context(tc.tile_pool(name="pw", bufs=_PBUFS, space="PSUM"))
        pcs = ctx.enter_context(tc.tile_pool(name="pcs", bufs=2, space="PSUM"))

        band_sb = cpool.tile([128, 4, _W], BF16, name="band_sb", tag="band_sb")
        ones_sb = cpool.tile([128, 1], BF16, name="ones_sb", tag="ones_sb")
        # per-partition accumulators, 5 slots per image:
        # 5i+0 sum(a), 5i+1 sum(sp), 5i+2 sum(ratio), 5i+3 sum(a*t4), 5i+4 sum(pm)
        acc = cpool.tile([128, 5 * _BPC], F32, name="acc", tag="acc")

        def colsum(src_t, col):
            """acc[:, col] = per-partition strided column sums of src_t via
            16 accumulated ones-matmuls on PE (full sum = host partition-sum)."""
            ps = pcs.tile([128, 1], F32, name="cs", tag="cs")
            flat = src_t[:].rearrange("p j w -> p (j w)")
            for c in range(16):
                nc.tensor.matmul(
                    out=ps[:], lhsT=flat[:, ts(c, 128)], rhs=ones_sb[:],
                    start=(c == 0), stop=(c == 15),
                )
            nc.vector.tensor_copy(acc[:, col : col + 1], ps[:])

        prev_cs = None

        for i in range(_BPC):
            # ---- loads (mb/pb first: they gate Pool's pm, the top engine).
            # Image 0: mb/pb arrive as interleaved j-chunk tiles so Pool's
            # first pm chunk starts ~1.5us in instead of waiting 512KB DMAs.
            mb = ipool.tile([128, 4, _W], BF16, name="mb", tag="mb")
            nc.sync.dma_start(mb[:], mask_d[i].rearrange("(j p) w -> p j w", p=128))
            pb = ipool.tile([128, 4, _W], BF16, name="pb", tag="pb")
            nc.sync.dma_start(pb[:], pred_d[i].rearrange("(j p) w -> p j w", p=128))
            mbj = [mb[:, j, :] for j in range(4)]
            pbj = [pb[:, j, :] for j in range(4)]
            mh = ipool.tile([128, 4, _W], BF16, name="mh", tag="mh")
            nc.sync.dma_start(mh[:], mh_d[i].rearrange("(j p) w -> p j w", p=128))
            if i == 0:
                nc.sync.dma_start(
                    band_sb[:], band_d.rearrange("(j p) c -> p j c", p=128)
                )
            m961 = ipool.tile([128, 4, _W], BF16, name="m961", tag="m961")
            nc.sync.dma_start(m961[:], m961_d[i].rearrange("(j p) w -> p j w", p=128))
            if i == 0:
                nc.sync.dma_start(ones_sb[:], ones_d)

            # ------------- box filter pass 1: W1 = B @ M (H direction) -------
            # W1p[p, jb, w] = sum_h B[h, jb*128+p] * M[h, w]
            w1p = pw.tile([128, 4, _W], F32, name="w1p", tag="w1p")
            for jb in range(4):
                js = [j for j in (jb - 1, jb, jb + 1) if 0 <= j < 4]
                for n, j in enumerate(js):
                    nc.tensor.matmul(
                        out=w1p[:, jb, :],
                        lhsT=band_sb[:, j, ts(jb, 128)],
                        rhs=mbj[j],
                        start=(n == 0),
                        stop=(n == len(js) - 1),
                    )

            # ----- pointwise + box pipeline, ordered for engine queues -----
            # Pool queue: pm -> hen-chunk -> Sdiff-main -> t1-half
            pm = mpool.tile([128, 4, _W], BF16, name="pm", tag="pm")
            e4 = mpool.tile([128, 4, _W], BF16, name="e4", tag="e4")
            tt_split(_SP_PM, pm, pb, mb, ALU.mult, img=i)
            # ACT chain: E -> sp ; den3 -> lnd -> rcp3 (den3 on DVE)
            nc.scalar.activation(e4[:], pb[:], AF.Exp)
            sp4 = mpool.tile([128, 4, _W], BF16, name="sp4", tag="sp4")
            nc.scalar.activation(
                sp4[:], e4[:], AF.Ln, bias=1.0,
                accum_out=acc[:, 5 * i + 1 : 5 * i + 2],
            )
            den3 = mpool.tile([128, 4, _W], BF16, name="den3", tag="den3")
            tt_split(_SP_DEN3, den3, e4, mh, ALU.add, img=i)
            lnd = mpool.tile([128, 4, _W], BF16, name="lnd", tag="lnd")
            nc.scalar.activation(lnd[:], den3[:], AF.Ln)
            rcp3 = mpool.tile([128, 4, _W], BF16, name="rcp3", tag="rcp3")
            nc.scalar.activation(rcp3[:], lnd[:], AF.Exp, scale=-1.0)
            hen = mpool.tile([128, 4, _W], BF16, name="hen", tag="hen")
            tt_split(_SP_HEN, hen, e4, mh, ALU.mult, img=i)

            # ------------- box filter pass 2: scan + shifted diffs -----------
            # S[p, (j,w)] = running sum of W1p along the flattened free dim.
            s4 = spool.tile([128, 4, _W], F32, name="s4", tag="s4")
            nc.vector.tensor_tensor_scan(
                s4[:].rearrange("p j w -> p (j w)"),
                w1p[:].rearrange("p j w -> p (j w)"),
                mb[:].rearrange("p j w -> p (j w)"),
                0.0,
                ALU.add,
                ALU.bypass,
            )
            # box[j, w] = S[j, w+KH] - S[j, w-KH-1]  (out-of-range terms:
            # right edge clamps to S[j, 511]; left edge uses S[j-1, 511],
            # which is the scan's carry into segment j, i.e. "S[j, -1]").
            box = mpool.tile([128, 4, _W], BF16, name="box", tag="box")
            sd_pool_img = _SDIFF_POOL and not (
                _SDIFF_TAIL_DVE and i >= _BPC - _TAIL_IMGS
            )
            # Sdiff main body, split at column KH+1+_SD_COLS (Pool | DVE)
            sdc = _SD_COLS if sd_pool_img else 0
            if sdc > 0:
                nc.gpsimd.tensor_tensor(
                    box[:, :, KH + 1 : KH + 1 + sdc],
                    s4[:, :, KW : KW + sdc],
                    s4[:, :, 0:sdc],
                    ALU.subtract,
                )
            if sdc < _W - KW:
                nc.vector.tensor_tensor(
                    box[:, :, KH + 1 + sdc : _W - KH],
                    s4[:, :, KW + sdc : _W],
                    s4[:, :, sdc : _W - KW],
                    ALU.subtract,
                )
            # left edge, j = 0: box = S[0, w+KH] (no carry; scan starts at 0)
            nc.vector.tensor_copy(box[:, 0:1, 0 : KH + 1], s4[:, 0:1, KH:KW])
            # left edge, j >= 1: box = S[j, w+KH] - S[j-1, 511]
            nc.vector.tensor_tensor(
                box[:, 1:4, 0 : KH + 1],
                s4[:, 1:4, KH:KW],
                s4[:, 0:3, _W - 1 : _W].broadcast_to([128, 3, KH + 1]),
                ALU.subtract,
            )
            # right edge: box = S[j, 511] - S[j, w-KH-1]
            nc.vector.tensor_tensor(
                box[:, :, _W - KH : _W],
                s4[:, :, _W - 1 : _W].broadcast_to([128, 4, KH]),
                s4[:, :, _W - KW : _W - KH - 1],
                ALU.subtract,
            )
            if prev_cs is not None:
                prev_cs()
                prev_cs = None
            # ratio = (hen + 0.5) * rcp3 = numB/den; its accum gives sum(ratio)
            ratio = mpool.tile([128, 4, _W], BF16, name="ratio", tag="hen")
            nc.vector.scalar_tensor_tensor(
                out=ratio[:], in0=hen[:], scalar=0.5, in1=rcp3[:],
                op0=ALU.add, op1=ALU.mult,
                accum_out=acc[:, 5 * i + 2 : 5 * i + 3],
            )
            # t1 = sp - pm ; t4 = t1 - ratio
            t1 = mpool.tile([128, 4, _W], BF16, name="t1", tag="t1")
            tt_split(_SP_T1, t1, sp4, pm, ALU.subtract, img=i)
            t4 = mpool.tile([128, 4, _W], BF16, name="t4", tag="t4")
            tt_split(_SP_T4, t4, t1, ratio, ALU.subtract, img=i)

            def emit_b(i=i, box=box, m961=m961, pm=pm, t4=t4):
                # xm = box - 961*M ;  a = |xm/961| via the Abs-activation scale
                xm = mpool.tile([128, 4, _W], BF16, name="xm", tag="xm")
                nc.vector.tensor_tensor(xm[:], box[:], m961[:], ALU.subtract)
                a4 = mpool.tile([128, 4, _W], BF16, name="a4", tag="a4")
                nc.scalar.activation(
                    a4[:], xm[:], AF.Abs, scale=1.0 / 961.0,
                    accum_out=acc[:, 5 * i : 5 * i + 1],
                )
                # a*t4 product; its sum (and pm's) via PE colsums
                at4 = mpool.tile([128, 4, _W], BF16, name="at4", tag="xm")
                nc.vector.tensor_tensor(at4[:], a4[:], t4[:], ALU.mult)
                colsum(pm, 5 * i + 4)
                colsum(at4, 5 * i + 3)

            prev_cs = emit_b

        prev_cs()
        nc.sync.dma_start(out_d[:], acc[:])

    nc.compile()
    _CACHE["nc"] = nc
    return nc


def _prep_inputs(pred, mask):
    import ml_dtypes

    bf16 = ml_dtypes.bfloat16
    p = np.asarray(pred, np.float32).reshape(_B, _H, _W)
    m = np.asarray(mask, np.float32).reshape(_B, _H, _W)
    pb = np.ascontiguousarray(p.astype(bf16))
    mb = np.ascontiguousarray(m.astype(bf16))
    m32 = mb.astype(np.float32)
    mh = np.ascontiguousarray(((m32 + 1.0) * 0.5).astype(bf16))
    m961 = np.ascontiguousarray((m32 * 961.0).astype(bf16))
    return pb, mb, mh, m961


def run_cores(pred, mask, trace=False, tmpdir=None):
    """Run the SPMD kernel; returns (list of per-core sum arrays, results)."""
    import ml_dtypes
    from concourse.bass_utils import run_bass_kernel_spmd

    nc = _build()
    pb, mb, mh, m961 = _prep_inputs(pred, mask)
    band = _band_np()
    ones = np.ones((128, 1), ml_dtypes.bfloat16)
    sl = lambda a, c: a[c * _BPC : (c + 1) * _BPC]
    in_maps = [
        {
            "pred": sl(pb, c),
            "mask": sl(mb, c),
            "mh": sl(mh, c),
            "m961": sl(m961, c),
            "band": band,
            "ones": ones,
        }
        for c in range(_NC)
    ]
    kw = {}
    if trace:
        kw = dict(trace=True, trace_cores=[0], tmpdir=tmpdir)
    br = run_bass_kernel_spmd(nc, in_maps, list(range(_NC)), **kw)
    outs = [
        br.results[c]["out"].reshape(128, 5 * _BPC).astype(np.float64).sum(axis=0)
        for c in range(_NC)
    ]
    return outs, br


def finish(outs):
    losses = []
    for c in range(_NC):
        o = outs[c]
        for i in range(_BPC):
            sa, ssp, srat, sat4, spm = o[5 * i : 5 * i + 5]
            st4 = ssp - spm - srat
            swt = sat4 + 0.2 * st4 + sa + 0.2 * _HW
            losses.append(5.0 * swt / (_HW + 5.0 * sa))
    return np.float32(np.mean(losses))


def kernel(pred, mask):
    outs, _ = run_cores(pred, mask)
    return finish(outs)
